# revision 30
# baseline (speedup 1.0000x reference)
"""Trainium2 Bass kernel for: cummax(W) ++ cummax(H) -> Linear(2C, C).

Reference semantics (shapes hardcoded):
    grid [16, 128, 128, 256] f32
    xc = cummax(grid, axis=2)   # along W
    yc = cummax(grid, axis=1)   # along H
    out = concat([xc, yc], -1) @ W[512, 256] + b[256]    # [16, 128, 128, 256]

Strategy: data-parallel over batch (2 batches / core on 8 cores).
Host pre-transposes grid to channels-first fp16 [c_half, c, b, h, w] so
on-chip tiles are [c(128 partitions), (c_half, b, h, w) free].  Both
per-core batches are processed together in h-row chunks (8 rows, with
half-size chunks at the ends to shrink the pipeline ramp and tail):
  - W-cummax: segmented max-scan (custom DVE op SEGMAX_ANT, row-shift
    trick) over the flat (b, h, w) dim -- 16 segments per op, with a
    hand-written 2X_1PORT uop program that runs 2 fp16 elems/cycle
    (2x the autogenerated 1x scan; see _segmax_uops_2x).
  - H-cummax: row-recurrence y[h] = max(y[h-1], g[h]) via fp16 DVE
    tensor_tensor at FD=512 (both c-halves x both batches per op).
  - Matmul: out[c_half, pix] += W_k[feat, c].T @ X_k[feat, pix], fp16
    operands, fp32 PSUM, 4 K-chunks (xc0, xc1, yc0, yc1), N=512 per MM.
    ~20 junk matmuls at kernel start warm the PE's HAM clock gate.
  - Bias is added during the ScalarE PSUM->SBUF copy; output is stored
    fp16 (host upcasts) to halve output DMA.
Everything fp16 on chip: monotone rounding commutes with cummax, and
the matmul accumulates in fp32, so rel err ~4e-4 vs the fp32 reference.
"""

import numpy as np

import concourse.tile as tile
from concourse import bacc, bass_isa, dve_ops, mybir
from concourse.bass_utils import run_bass_kernel_spmd
from concourse.dve_ops import DveOp
from concourse.dve_spec import AluOp, Spec, Src0, Src1, lower, scan
from concourse.dve_table_gen import dve_ver_for
from concourse.dve_uop import (
    AluInp,
    DelayInp,
    DveOpSpec,
    InpSel,
    OutPath,
    OutSel,
    Trigger,
    UopConfig,
    UopDpConfig,
)


def _segmax_uops_2x():
    """Hand-written 2X_1PORT uop program for the segmented max-scan.

    In 2X_1PORT mode the engine reads two consecutive fp16 elements per
    cycle per port (SRC_0/SRC_0_HI = data pair, SRC_1/SRC_1_HI = shift
    pair) and writes a packed pair (WR0_LO/WR0_HI).  Datapath per pair
    (xe, xo) with shifts (she, sho):
        A = xe + she; B = xo + sho          (blk0, blk1)
        C = max(A, B)                       (blk2, pair max)
        S = max(S, C)                       (blk3, running max, self-loop;
                                             its pre-update value S_prev is
                                             captured into a delay lane via
                                             DelayInp.CURR_ALU_OUT = "this
                                             block's flop, previous cycle")
        xc_e = max(S_prev, A)               (blk4)
        xc_o = max(xc_e, B)                 (blk5)
        OUT_e = xc_e - she                  (blk6 -> delay lane 0)
        OUT_o = xc_o - sho                  (blk7 -> ALU out)
    Write paths mirror the stock tensor_mask 2x program (slot 105):
    WR0_LO <- DELAY_0 (even), WR0_HI <- ALU_OUT (odd).
    """
    A, D, Op = AluInp, DelayInp, AluOp
    bs = [UopDpConfig() for _ in range(8)]
    # chains: c0=xe, c1=she, c2=xo, c3=sho, c4=MAX_NEG (seed const)
    bs[0].enable_alu(Op.ADD, A.PREV_DELAY_0, A.PREV_DELAY_1)
    bs[0].pass_through_delay(1, 2, 3, 4)
    bs[1].enable_alu(Op.ADD, A.PREV_DELAY_2, A.PREV_DELAY_3)
    bs[1].enable_delay_from_src(D.PREV_ALU_OUT, 0)  # c0 <- A
    bs[1].pass_through_delay(1, 3, 4)
    bs[2].enable_alu(Op.MAX, A.PREV_ALU_OUT, A.PREV_DELAY_0)  # C = max(B, A)
    bs[2].enable_delay_from_src(D.PREV_ALU_OUT, 2)  # c2 <- B
    bs[2].pass_through_delay(0, 1, 3, 4)
    bs[3].enable_alu(Op.MAX, A.CURR_ALU_OUT, A.PREV_ALU_OUT)  # S = max(S, C)
    bs[3].enable_delay_from_src(D.CURR_ALU_OUT, 4)  # c4 <- S_prev
    bs[3].pass_through_delay(0, 1, 2, 3)
    bs[4].enable_alu(Op.MAX, A.PREV_DELAY_4, A.PREV_DELAY_0)  # xc_e
    bs[4].pass_through_delay(1, 2, 3)
    bs[5].enable_alu(Op.MAX, A.PREV_ALU_OUT, A.PREV_DELAY_2)  # xc_o
    bs[5].enable_delay_from_src(D.PREV_ALU_OUT, 0)  # c0 <- xc_e
    bs[5].pass_through_delay(1, 3)
    bs[6].enable_alu(Op.SUBTRACT, A.PREV_DELAY_0, A.PREV_DELAY_1)  # OUT_e
    bs[6].enable_delay_from_src(D.PREV_ALU_OUT, 2)  # c2 <- xc_o
    bs[6].pass_through_delay(3)
    bs[7].enable_alu(Op.SUBTRACT, A.PREV_DELAY_2, A.PREV_DELAY_3)  # OUT_o
    bs[7].enable_delay_from_src(D.PREV_ALU_OUT, 0)  # c0 <- OUT_e

    inp = [
        InpSel.ZERO,
        InpSel.SRC_0,
        InpSel.SRC_1,
        InpSel.SRC_0_HI,
        InpSel.SRC_1_HI,
        InpSel.MAX_NEG,
        InpSel.ZERO,
        InpSel.ZERO,
    ]
    inp_en = [0, 1, 1, 1, 1, 1, 0, 0]
    steady = UopConfig(
        inp=inp,
        inp_enable=inp_en,
        out={
            OutPath.WR0_LO: OutSel.DELAY_0,
            OutPath.WR0_HI: OutSel.ALU_OUT,
            OutPath.WR1_LO: OutSel.ALU_OUT,
            OutPath.WR1_HI: OutSel.ALU_OUT,
        },
        out_enable={
            OutPath.WR0_LO: 1,
            OutPath.WR0_HI: 1,
            OutPath.WR1_LO: 0,
            OutPath.WR1_HI: 0,
        },
        require_inp0=1,
        require_inp1=1,
        trigger=(Trigger.SRC_TENSOR_DONE, Trigger.NONE, Trigger.NONE),
        next_uop=(0, 0, 0),
        repeat_count=0,
        datapath_config=bs,
    )
    # Seed: one cycle, no src consumed; MAX_NEG rides chain 4 to blk3 whose
    # BYPASS initialises the running-max flop (mirrors the 1x seed).
    sd = [UopDpConfig() for _ in range(8)]
    for k in range(3):
        sd[k].pass_through_delay(4)
    sd[3].enable_alu(Op.BYPASS, A.PREV_DELAY_4, A.PREV_DELAY_4)
    seed = UopConfig(
        inp=inp,
        inp_enable=inp_en,
        out={o: OutSel.ALU_OUT for o in OutPath},
        out_enable={o: 0 for o in OutPath},
        require_inp0=0,
        require_inp1=0,
        trigger=(Trigger.COUNT, Trigger.NONE, Trigger.NONE),
        next_uop=(1, 0, 0),
        repeat_count=1,
        datapath_config=sd,
    )
    return [seed, steady]


def _register_segmax():
    """Custom DVE op: segmented max-scan via the row-shift trick.

    body = scan(max, x + shift) - shift, where shift[row r] = r*BIG.  With
    BIG > 2*max|x| the running max can never cross a row boundary (later
    rows' shifted values strictly dominate), so one continuous scan yields
    an independent cummax per row.  The hand-written 2X_1PORT variant
    (see _segmax_uops_2x) processes 2 fp16 elems/cycle vs 1 for the
    autogenerated 1x program; the engine falls back to 1x whenever the
    runtime mem-pattern check fails, so 2x is purely a fast path.
    """
    name = "SEGMAX_ANT"
    if name in dve_ops._SUB_OPCODE_FOR_NAME:
        return next(o for o in dve_ops.OPS if o.name == name)
    ver = dve_ver_for("TRN2")
    row = dve_ops._CUSTOM_DVE_ROW_BASE + len(dve_ops.OPS)
    spec = Spec(
        body=scan(AluOp.MAX, Src0 + Src1) - Src1,
        reference=lambda in0, in1, s0, s1, imm2: (
            np.maximum.accumulate(
                (in0.astype(np.float32) + in1).reshape(in0.shape[0], -1), axis=1
            ).reshape(in0.shape)
            - in1
        ),
    )
    tmp = DveOpSpec(
        name=name,
        opcode=row,
        uops=lower(spec, ver=ver),
        uops_2x=_segmax_uops_2x(),
        rd1_en=True,
        perf_max=1,
    )
    tmp.validate(ver)
    op = DveOp(name, spec, subdim=False, uops_sha={ver: tmp.sha(ver)})
    # Seed the compile cache so DveOp.compile() returns the spec WITH the
    # 2x variant (it would otherwise re-lower the 1x-only program and fail
    # the sha pin).
    dve_ops._COMPILE_CACHE[(name, ver)] = tmp
    dve_ops.OPS.append(op)
    dve_ops.CUSTOM_DVE_SPECS[name] = spec
    dve_ops._SUB_OPCODE_FOR_NAME[name] = row
    return op


SEGMAX = _register_segmax()


def _emit_segmax_2x(nc, out, in0, in1):
    """nc.vector._custom_dve(SEGMAX, ...) with perf_max=1 in byte-36 so the
    engine may select the 2X_1PORT uop program (it re-checks dtype/stride/
    alignment at runtime and silently falls back to 1x)."""
    v = nc.vector
    if SEGMAX.name not in v.bass.m.ant_custom_dve_ops:
        v.bass.m.ant_custom_dve_ops = sorted(
            {*v.bass.m.ant_custom_dve_ops, SEGMAX.name}
        )
    shape = bass_isa.CustomDveShape.TTSS  # 2D in1 -> 1D free pattern
    isa_opcode = v.bass.isa.Opcode[
        f"NEURON_ISA_TPB_OPCODE_CUSTOM_DVE_ANT_{shape.slot()}"
    ].value
    zero = mybir.ImmediateValue(dtype=mybir.dt.float32, value=0.0)
    ins = [
        v.lower_ap(in0, for_isa=True, opt=True),
        v.lower_ap(in1, for_isa=True, opt=True),
        zero,
        zero,
    ]
    outs = [v.lower_ap(out, for_isa=True, opt=True)]
    return v.add_instruction(
        bass_isa.InstCustomDveAnt(
            name=v.bass.get_next_instruction_name(),
            op_name=SEGMAX.name,
            rd1_en=True,
            subdim=0,
            imm2=0.0,
            shape=shape,
            row=dve_ops.get_dve_sub_opcode(SEGMAX.name),
            isa_opcode=isa_opcode,
            ins=ins,
            outs=outs,
            perf_max=1,
        )
    )

B, H, W_DIM, C = 16, 128, 128, 256
N_CORES = 8
NB = B // N_CORES          # batches per core
P = 128                    # partitions
NCH = C // P               # channel halves (2)
KT = (2 * C) // P          # K chunks for the matmul (4)
HC = 8                     # h rows per streamed chunk (both batches)
NEG = -1.0e38

FP32 = mybir.dt.float32
FP16 = mybir.dt.float16


def build(nb=NB, h=H, hc=HC):
    """Build the per-core Bass program (same program on all cores)."""
    Alu = mybir.AluOpType
    # Chunk row counts: half-size chunks at both ends shrink the pipeline
    # ramp (first scan starts sooner) and tail (last chunk's matmul+drain+
    # store is shorter).
    # Note: chunk sizes must keep nb * hc_j * W_DIM >= 1024 (one PSUM group).
    if h > 4 * hc:
        half = hc // 2
        chunks = [half, half] + [hc] * ((h - 2 * hc) // hc) + [half, half]
    else:
        chunks = [hc] * (h // hc)
    assert sum(chunks) == h
    hw = h * W_DIM

    nc = bacc.Bacc("TRN2", target_bir_lowering=False, debug=False)
    grid_t = nc.dram_tensor(
        "grid_t", [NCH, P, nb, h, W_DIM], FP16, kind="ExternalInput"
    ).ap()
    # Weights pre-arranged on host as [f, ch*KT, c] so the DMA reads one
    # contiguous 2 KiB run per partition (256 B runs pay a 2x DMA penalty).
    w_t = nc.dram_tensor("w_t", [P, NCH * KT, P], FP16, kind="ExternalInput").ap()
    b_t = nc.dram_tensor("b_t", [P, NCH], FP32, kind="ExternalInput").ap()
    out_t = nc.dram_tensor(
        "out_t", [NCH, P, nb, hw], FP16, kind="ExternalOutput"
    ).ap()

    with tile.TileContext(nc) as tc:
        with (
            tc.tile_pool(name="consts", bufs=1) as consts,
            tc.tile_pool(name="gin", bufs=6) as gin,
            tc.tile_pool(name="xcp", bufs=4) as xcp,
            tc.tile_pool(name="ycp", bufs=4) as ycp,
            tc.tile_pool(name="outp", bufs=4) as outp,
            tc.tile_pool(name="psum", bufs=4, space="PSUM") as psump,
        ):
            # Chunk-0 grid DMA issued FIRST so the first scan's data is in
            # flight before the consts DMAs queue behind it.
            g0 = gin.tile([P, NCH, nb, chunks[0], W_DIM], FP16)
            for ch in range(NCH):
                nc.sync.dma_start(
                    out=g0[:, ch], in_=grid_t[ch][:, :, 0 : chunks[0], :]
                )
            # PE warm-up: the HAM clock gate holds the PE at ~half rate until
            # ~4 us of sustained activity.  Junk matmuls on a zeroed tile
            # warm it during the initial DMA wait so the real matmul stream
            # starts at full rate.
            warm = consts.tile([P, 512], FP16)
            nc.gpsimd.memset(warm, 0.0)
            wpt = psump.tile([P, 1024], FP32, tag="pt")
            for _ in range(20):
                nc.tensor.matmul(
                    wpt[:, 0:512], warm[:, 0:128], warm, start=True, stop=True
                )
            # Weights as 8 stationary [feat, c] tiles, indexed ch_out*KT + k.
            w_sb = consts.tile([P, NCH * KT, P], FP16)
            nc.sync.dma_start(out=w_sb, in_=w_t)
            b_sb = consts.tile([P, NCH], FP32)
            nc.sync.dma_start(out=b_sb, in_=b_t)
            # ACT touches b_sb once so later drains never carry the DMA wait
            # (the activation struct has a single sync-wait slot).
            b_scratch = consts.tile([P, NCH], FP32)
            nc.scalar.copy(out=b_scratch, in_=b_sb)
            # Row-shift tile for the custom segmented max-scan: row r gets
            # r*BIG (BIG > 2*max|x| keeps rows from bleeding into each other).
            # Built on gpsimd (otherwise idle).  Only 16 rows: half chunks
            # scan both c-halves at once (16 segments), full chunks scan one
            # c-half per op (16 segments each) -- so every scan is a uniform
            # [P, 2048] op over the same shift tile, and the first scan only
            # waits for 16 memsets (~3 us) instead of 32.
            n_rows = 16
            shift = consts.tile([P, n_rows, W_DIM], FP16)
            for r in range(n_rows):
                nc.gpsimd.memset(shift[:, r, :], r * 16.0)
            shift_f = shift.rearrange("p r w -> p (r w)")

            y_prev = None
            hc_prev = 0
            row0 = 0
            for j, hc_j in enumerate(chunks):
                cpix = nb * hc_j * W_DIM
                if j == 0:
                    g = g0
                else:
                    g = gin.tile([P, NCH, nb, hc_j, W_DIM], FP16)
                    for ch in range(NCH):
                        nc.sync.dma_start(
                            out=g[:, ch],
                            in_=grid_t[ch][:, :, row0 : row0 + hc_j, :],
                        )
                g_f = g.rearrange("p c b h w -> p c (b h w)")

                # --- W-cummax: segmented scan along the flat (b, h, w) dim.
                x = xcp.tile([P, NCH, cpix], FP16)
                # The scan lowers to S2S2D2_STT, which has very few
                # sync-wait slots; absorb the cross-engine waits (g DMA,
                # x slot release) into a cheap DVE copy first.
                nc.vector.tensor_copy(x[:, :, 0:1], g_f[:, :, 0:1])
                if NCH * cpix == n_rows * W_DIM:
                    # half chunk: both c-halves in one 16-segment scan
                    _emit_segmax_2x(
                        nc,
                        out=x.rearrange("p c x -> p (c x)"),
                        in0=g_f.rearrange("p c x -> p (c x)"),
                        in1=shift_f,
                    )
                else:
                    # full chunk: one 16-segment scan per c-half
                    for ch in range(NCH):
                        _emit_segmax_2x(
                            nc, out=x[:, ch, :], in0=g_f[:, ch, :], in1=shift_f
                        )

                # --- H-cummax: row recurrence (c-halves x batches per op).
                y = ycp.tile([P, NCH, nb, hc_j, W_DIM], FP16)
                for hh in range(hc_j):
                    if j == 0 and hh == 0:
                        nc.vector.tensor_copy(y[:, :, :, 0, :], g[:, :, :, 0, :])
                    else:
                        prev = (
                            y[:, :, :, hh - 1, :]
                            if hh > 0
                            else y_prev[:, :, :, hc_prev - 1, :]
                        )
                        nc.vector.tensor_tensor(
                            y[:, :, :, hh, :], g[:, :, :, hh, :], prev, Alu.max
                        )
                y_prev = y
                hc_prev = hc_j
                y_f = y.rearrange("p c b h w -> p c (b h w)")

                # --- Matmul + bias. K order: xc0, xc1, yc0, yc1.
                # o has a 1-element pad: an ACT "touch" writes it so the
                # slot-release hazard (out-DMA) lands on the ACT clock
                # without overlapping the drains' writes (overlapping
                # same-engine WAW costs an extra sync-wait slot).
                o_raw = outp.tile([P, NCH * cpix + 1], FP16)
                nc.scalar.copy(
                    out=o_raw[:, NCH * cpix : NCH * cpix + 1],
                    in_=w_sb[:, 0, 0:1],
                )
                o = o_raw[:, 0 : NCH * cpix].rearrange("p (c x) -> p c x", c=NCH)
                for s in range(cpix // 1024):
                    for ch_out in range(NCH):
                        pt = psump.tile([P, 1024], FP32, tag="pt")
                        # N=1 dummy matmul: re-points the PSUM slot-release
                        # hazard (ACT) onto the PE clock, so real matmuls
                        # and the drain each carry a single sync wait (the
                        # LDW/AC structs allow only one).  The first one
                        # also absorbs the w_sb DMA wait.
                        nc.tensor.matmul(
                            pt[:, 0:1],
                            w_sb[:, 0, :],
                            w_sb[:, 0, 0:1],
                            start=True,
                            stop=True,
                        )
                        for pg in range(2):
                            lo = s * 1024 + pg * 512
                            for k in range(KT):
                                src = x if k < NCH else y_f
                                rhs = src[:, k % NCH, lo : lo + 512]
                                nc.tensor.matmul(
                                    pt[:, pg * 512 : (pg + 1) * 512],
                                    w_sb[:, ch_out * KT + k, :],
                                    rhs,
                                    start=(k == 0),
                                    stop=(k == KT - 1),
                                )
                        nc.scalar.activation(
                            out=o[:, ch_out, s * 1024 : (s + 1) * 1024],
                            in_=pt,
                            func=mybir.ActivationFunctionType.Identity,
                            bias=b_sb[:, ch_out : ch_out + 1],
                            scale=1.0,
                        )

                o_4d = o.rearrange("p c (b x) -> p c b x", b=nb)
                for ch in range(NCH):
                    nc.sync.dma_start(
                        out=out_t[ch][
                            :, :, row0 * W_DIM : (row0 + hc_j) * W_DIM
                        ],
                        in_=o_4d[:, ch],
                    )
                row0 += hc_j
    nc.compile()
    return nc


_built = {}


def _get_nc():
    if "nc" not in _built:
        _built["nc"] = build()
    return _built["nc"]


def make_in_maps(grid, Wm, bv):
    """Host-side shard + layout transform. Returns per-core input maps."""
    grid = np.asarray(grid, dtype=np.float32).astype(np.float16)
    Wm = np.asarray(Wm, dtype=np.float32).astype(np.float16)
    bv = np.asarray(bv, dtype=np.float32)
    # [f=P, ch*KT, c]: w_t[f, ch*KT+k, c] = W[k*P+f, ch*P+c] -- one
    # contiguous 2 KiB run per partition for the DMA.
    w_t = np.ascontiguousarray(
        Wm.reshape(KT, P, NCH, P).transpose(1, 2, 0, 3).reshape(P, NCH * KT, P)
    )
    b_t = np.ascontiguousarray(bv.reshape(NCH, P).T)
    in_maps = []
    for i in range(N_CORES):
        gc = grid[i * NB : (i + 1) * NB]  # [NB, H, W, C]
        # -> [ch, c, b, h, w]
        gt = np.ascontiguousarray(gc.transpose(3, 0, 1, 2)).reshape(
            NCH, P, NB, H, W_DIM
        )
        in_maps.append({"grid_t": gt, "w_t": w_t, "b_t": b_t})
    return in_maps


def assemble_output(results):
    """Per-core [NCH, P, NB, H*W] fp16 -> full [B, H, W, C] fp32."""
    outs = []
    for i in range(N_CORES):
        ot = results[i]["out_t"]
        oc = (
            ot.reshape(NCH, P, NB, H, W_DIM)
            .transpose(2, 3, 4, 0, 1)
            .reshape(NB, H, W_DIM, C)
            .astype(np.float32)
        )
        outs.append(oc)
    return np.ascontiguousarray(np.concatenate(outs, axis=0))


def run(inputs, **kwargs):
    """Run on hardware; returns (output, BassKernelResults)."""
    nc = _get_nc()
    in_maps = make_in_maps(inputs["grid"], inputs["W"], inputs["b"])
    res = run_bass_kernel_spmd(nc, in_maps, core_ids=list(range(N_CORES)), **kwargs)
    return assemble_output(res.results), res


def kernel(**inputs) -> np.ndarray:
    out, _ = run(inputs)
    return out



# revision 31
# speedup vs baseline: 1.0914x; 1.0914x over previous
"""Trainium2 Bass kernel for: cummax(W) ++ cummax(H) -> Linear(2C, C).

Reference semantics (shapes hardcoded):
    grid [16, 128, 128, 256] f32
    xc = cummax(grid, axis=2)   # along W
    yc = cummax(grid, axis=1)   # along H
    out = concat([xc, yc], -1) @ W[512, 256] + b[256]    # [16, 128, 128, 256]

Strategy: data-parallel over batch (2 batches / core on 8 cores).
Host pre-transposes grid to channels-first fp16 [c_half, c, b, h, w] so
on-chip tiles are [c(128 partitions), (c_half, b, h, w) free].  Both
per-core batches are processed together in h-row chunks (8 rows, with
half-size chunks at the ends to shrink the pipeline ramp and tail):
  - W-cummax: segmented max-scan (custom DVE op SEGMAX_ANT, row-shift
    trick) over the flat (b, h, w) dim -- 16 segments per op, with a
    hand-written 2X_1PORT uop program that runs 2 fp16 elems/cycle
    (2x the autogenerated 1x scan; see _segmax_uops_2x).
  - H-cummax: row-recurrence y[h] = max(y[h-1], g[h]) via fp16 DVE
    tensor_tensor at FD=512 (both c-halves x both batches per op).
  - Matmul: out[c_half, pix] += W_k[feat, c].T @ X_k[feat, pix], fp16
    operands, fp32 PSUM, 4 K-chunks (xc0, xc1, yc0, yc1), N=512 per MM.
    ~20 junk matmuls at kernel start warm the PE's HAM clock gate.
  - Bias is added during the ScalarE PSUM->SBUF copy; output is stored
    fp16 (host upcasts) to halve output DMA.
Everything fp16 on chip: monotone rounding commutes with cummax, and
the matmul accumulates in fp32, so rel err ~4e-4 vs the fp32 reference.
"""

import numpy as np

import concourse.tile as tile
from concourse import bacc, bass_isa, dve_ops, mybir
from concourse.bass_utils import run_bass_kernel_spmd
from concourse.dve_ops import DveOp
from concourse.dve_spec import AluOp, Spec, Src0, Src1, lower, scan
from concourse.dve_table_gen import dve_ver_for
from concourse.dve_uop import (
    AluInp,
    DelayInp,
    DveOpSpec,
    InpSel,
    OutPath,
    OutSel,
    Trigger,
    UopConfig,
    UopDpConfig,
)


def _segmax_uops_2x():
    """Hand-written 2X_1PORT uop program for the segmented max-scan.

    In 2X_1PORT mode the engine reads two consecutive fp16 elements per
    cycle per port (SRC_0/SRC_0_HI = data pair, SRC_1/SRC_1_HI = shift
    pair) and writes a packed pair (WR0_LO/WR0_HI).  Datapath per pair
    (xe, xo) with shifts (she, sho):
        A = xe + she; B = xo + sho          (blk0, blk1)
        C = max(A, B)                       (blk2, pair max)
        S = max(S, C)                       (blk3, running max, self-loop;
                                             its pre-update value S_prev is
                                             captured into a delay lane via
                                             DelayInp.CURR_ALU_OUT = "this
                                             block's flop, previous cycle")
        xc_e = max(S_prev, A)               (blk4)
        xc_o = max(xc_e, B)                 (blk5)
        OUT_e = xc_e - she                  (blk6 -> delay lane 0)
        OUT_o = xc_o - sho                  (blk7 -> ALU out)
    Write paths mirror the stock tensor_mask 2x program (slot 105):
    WR0_LO <- DELAY_0 (even), WR0_HI <- ALU_OUT (odd).
    """
    A, D, Op = AluInp, DelayInp, AluOp
    bs = [UopDpConfig() for _ in range(8)]
    # chains: c0=xe, c1=she, c2=xo, c3=sho, c4=MAX_NEG (seed const)
    bs[0].enable_alu(Op.ADD, A.PREV_DELAY_0, A.PREV_DELAY_1)
    bs[0].pass_through_delay(1, 2, 3, 4)
    bs[1].enable_alu(Op.ADD, A.PREV_DELAY_2, A.PREV_DELAY_3)
    bs[1].enable_delay_from_src(D.PREV_ALU_OUT, 0)  # c0 <- A
    bs[1].pass_through_delay(1, 3, 4)
    bs[2].enable_alu(Op.MAX, A.PREV_ALU_OUT, A.PREV_DELAY_0)  # C = max(B, A)
    bs[2].enable_delay_from_src(D.PREV_ALU_OUT, 2)  # c2 <- B
    bs[2].pass_through_delay(0, 1, 3, 4)
    bs[3].enable_alu(Op.MAX, A.CURR_ALU_OUT, A.PREV_ALU_OUT)  # S = max(S, C)
    bs[3].enable_delay_from_src(D.CURR_ALU_OUT, 4)  # c4 <- S_prev
    bs[3].pass_through_delay(0, 1, 2, 3)
    bs[4].enable_alu(Op.MAX, A.PREV_DELAY_4, A.PREV_DELAY_0)  # xc_e
    bs[4].pass_through_delay(1, 2, 3)
    bs[5].enable_alu(Op.MAX, A.PREV_ALU_OUT, A.PREV_DELAY_2)  # xc_o
    bs[5].enable_delay_from_src(D.PREV_ALU_OUT, 0)  # c0 <- xc_e
    bs[5].pass_through_delay(1, 3)
    bs[6].enable_alu(Op.SUBTRACT, A.PREV_DELAY_0, A.PREV_DELAY_1)  # OUT_e
    bs[6].enable_delay_from_src(D.PREV_ALU_OUT, 2)  # c2 <- xc_o
    bs[6].pass_through_delay(3)
    bs[7].enable_alu(Op.SUBTRACT, A.PREV_DELAY_2, A.PREV_DELAY_3)  # OUT_o
    bs[7].enable_delay_from_src(D.PREV_ALU_OUT, 0)  # c0 <- OUT_e

    inp = [
        InpSel.ZERO,
        InpSel.SRC_0,
        InpSel.SRC_1,
        InpSel.SRC_0_HI,
        InpSel.SRC_1_HI,
        InpSel.MAX_NEG,
        InpSel.ZERO,
        InpSel.ZERO,
    ]
    inp_en = [0, 1, 1, 1, 1, 1, 0, 0]
    steady = UopConfig(
        inp=inp,
        inp_enable=inp_en,
        out={
            OutPath.WR0_LO: OutSel.DELAY_0,
            OutPath.WR0_HI: OutSel.ALU_OUT,
            OutPath.WR1_LO: OutSel.ALU_OUT,
            OutPath.WR1_HI: OutSel.ALU_OUT,
        },
        out_enable={
            OutPath.WR0_LO: 1,
            OutPath.WR0_HI: 1,
            OutPath.WR1_LO: 0,
            OutPath.WR1_HI: 0,
        },
        require_inp0=1,
        require_inp1=1,
        trigger=(Trigger.SRC_TENSOR_DONE, Trigger.NONE, Trigger.NONE),
        next_uop=(0, 0, 0),
        repeat_count=0,
        datapath_config=bs,
    )
    # Seed: one cycle, no src consumed; MAX_NEG rides chain 4 to blk3 whose
    # BYPASS initialises the running-max flop (mirrors the 1x seed).
    sd = [UopDpConfig() for _ in range(8)]
    for k in range(3):
        sd[k].pass_through_delay(4)
    sd[3].enable_alu(Op.BYPASS, A.PREV_DELAY_4, A.PREV_DELAY_4)
    seed = UopConfig(
        inp=inp,
        inp_enable=inp_en,
        out={o: OutSel.ALU_OUT for o in OutPath},
        out_enable={o: 0 for o in OutPath},
        require_inp0=0,
        require_inp1=0,
        trigger=(Trigger.COUNT, Trigger.NONE, Trigger.NONE),
        next_uop=(1, 0, 0),
        repeat_count=1,
        datapath_config=sd,
    )
    return [seed, steady]


def _register_segmax():
    """Custom DVE op: segmented max-scan via the row-shift trick.

    body = scan(max, x + shift) - shift, where shift[row r] = r*BIG.  With
    BIG > 2*max|x| the running max can never cross a row boundary (later
    rows' shifted values strictly dominate), so one continuous scan yields
    an independent cummax per row.  The hand-written 2X_1PORT variant
    (see _segmax_uops_2x) processes 2 fp16 elems/cycle vs 1 for the
    autogenerated 1x program; the engine falls back to 1x whenever the
    runtime mem-pattern check fails, so 2x is purely a fast path.
    """
    name = "SEGMAX_ANT"
    if name in dve_ops._SUB_OPCODE_FOR_NAME:
        return next(o for o in dve_ops.OPS if o.name == name)
    ver = dve_ver_for("TRN2")
    row = dve_ops._CUSTOM_DVE_ROW_BASE + len(dve_ops.OPS)
    spec = Spec(
        body=scan(AluOp.MAX, Src0 + Src1) - Src1,
        reference=lambda in0, in1, s0, s1, imm2: (
            np.maximum.accumulate(
                (in0.astype(np.float32) + in1).reshape(in0.shape[0], -1), axis=1
            ).reshape(in0.shape)
            - in1
        ),
    )
    tmp = DveOpSpec(
        name=name,
        opcode=row,
        uops=lower(spec, ver=ver),
        uops_2x=_segmax_uops_2x(),
        rd1_en=True,
        perf_max=1,
    )
    tmp.validate(ver)
    op = DveOp(name, spec, subdim=False, uops_sha={ver: tmp.sha(ver)})
    # Seed the compile cache so DveOp.compile() returns the spec WITH the
    # 2x variant (it would otherwise re-lower the 1x-only program and fail
    # the sha pin).
    dve_ops._COMPILE_CACHE[(name, ver)] = tmp
    dve_ops.OPS.append(op)
    dve_ops.CUSTOM_DVE_SPECS[name] = spec
    dve_ops._SUB_OPCODE_FOR_NAME[name] = row
    return op


SEGMAX = _register_segmax()


def _emit_segmax_2x(nc, out, in0, in1):
    """nc.vector._custom_dve(SEGMAX, ...) with perf_max=1 in byte-36 so the
    engine may select the 2X_1PORT uop program (it re-checks dtype/stride/
    alignment at runtime and silently falls back to 1x)."""
    v = nc.vector
    if SEGMAX.name not in v.bass.m.ant_custom_dve_ops:
        v.bass.m.ant_custom_dve_ops = sorted(
            {*v.bass.m.ant_custom_dve_ops, SEGMAX.name}
        )
    shape = bass_isa.CustomDveShape.TTSS  # 2D in1 -> 1D free pattern
    isa_opcode = v.bass.isa.Opcode[
        f"NEURON_ISA_TPB_OPCODE_CUSTOM_DVE_ANT_{shape.slot()}"
    ].value
    zero = mybir.ImmediateValue(dtype=mybir.dt.float32, value=0.0)
    ins = [
        v.lower_ap(in0, for_isa=True, opt=True),
        v.lower_ap(in1, for_isa=True, opt=True),
        zero,
        zero,
    ]
    outs = [v.lower_ap(out, for_isa=True, opt=True)]
    return v.add_instruction(
        bass_isa.InstCustomDveAnt(
            name=v.bass.get_next_instruction_name(),
            op_name=SEGMAX.name,
            rd1_en=True,
            subdim=0,
            imm2=0.0,
            shape=shape,
            row=dve_ops.get_dve_sub_opcode(SEGMAX.name),
            isa_opcode=isa_opcode,
            ins=ins,
            outs=outs,
            perf_max=1,
        )
    )

B, H, W_DIM, C = 16, 128, 128, 256
N_CORES = 8
NB = B // N_CORES          # batches per core
P = 128                    # partitions
NCH = C // P               # channel halves (2)
KT = (2 * C) // P          # K chunks for the matmul (4)
HC = 8                     # h rows per streamed chunk (both batches)
NEG = -1.0e38

FP32 = mybir.dt.float32
FP16 = mybir.dt.float16


def build(nb=NB, h=H, hc=HC):
    """Build the per-core Bass program (same program on all cores)."""
    Alu = mybir.AluOpType
    # Chunk row counts: half-size chunks at both ends shrink the pipeline
    # ramp (first scan starts sooner) and tail (last chunk's matmul+drain+
    # store is shorter).
    # Note: chunk sizes must keep nb * hc_j * W_DIM >= 1024 (one PSUM group).
    if h > 4 * hc:
        half = hc // 2
        chunks = [half, half] + [hc] * ((h - 2 * hc) // hc) + [half, half]
    else:
        chunks = [hc] * (h // hc)
    assert sum(chunks) == h
    hw = h * W_DIM

    nc = bacc.Bacc("TRN2", target_bir_lowering=False, debug=False)
    grid_t = nc.dram_tensor(
        "grid_t", [NCH, P, nb, h, W_DIM], FP16, kind="ExternalInput"
    ).ap()
    # Weights pre-arranged on host as [f, ch*KT, c] so the DMA reads one
    # contiguous 2 KiB run per partition (256 B runs pay a 2x DMA penalty).
    w_t = nc.dram_tensor("w_t", [P, NCH * KT, P], FP16, kind="ExternalInput").ap()
    b_t = nc.dram_tensor("b_t", [P, NCH], FP32, kind="ExternalInput").ap()
    out_t = nc.dram_tensor(
        "out_t", [NCH, P, nb, hw], FP16, kind="ExternalOutput"
    ).ap()

    with tile.TileContext(nc) as tc:
        with (
            tc.tile_pool(name="consts", bufs=1) as consts,
            tc.tile_pool(name="gin", bufs=4) as gin,
            tc.tile_pool(name="xcp", bufs=3) as xcp,
            tc.tile_pool(name="ycp", bufs=3) as ycp,
            tc.tile_pool(name="outp", bufs=3) as outp,
            tc.tile_pool(name="psum", bufs=4, space="PSUM") as psump,
        ):
            # Chunk-0 grid DMA issued FIRST so the first scan's data is in
            # flight before the consts DMAs queue behind it.
            g0 = gin.tile([P, NCH, nb, chunks[0], W_DIM], FP16)
            for ch in range(NCH):
                nc.sync.dma_start(
                    out=g0[:, ch], in_=grid_t[ch][:, :, 0 : chunks[0], :]
                )
            # PE warm-up: the HAM clock gate holds the PE at ~half rate until
            # ~4 us of sustained activity.  Junk matmuls on a zeroed tile
            # warm it during the initial DMA wait so the real matmul stream
            # starts at full rate.
            warm = consts.tile([P, 512], FP16)
            nc.gpsimd.memset(warm, 0.0)
            wpt = psump.tile([P, 1024], FP32, tag="pt")
            for _ in range(20):
                nc.tensor.matmul(
                    wpt[:, 0:512], warm[:, 0:128], warm, start=True, stop=True
                )
            # Weights as 8 stationary [feat, c] tiles, indexed ch_out*KT + k.
            w_sb = consts.tile([P, NCH * KT, P], FP16)
            nc.sync.dma_start(out=w_sb, in_=w_t)
            b_sb = consts.tile([P, NCH], FP32)
            nc.sync.dma_start(out=b_sb, in_=b_t)
            # ACT touches b_sb once so later drains never carry the DMA wait
            # (the activation struct has a single sync-wait slot).
            b_scratch = consts.tile([P, NCH], FP32)
            nc.scalar.copy(out=b_scratch, in_=b_sb)
            # Row-shift tile for the custom segmented max-scan: row r gets
            # r*BIG (BIG > 2*max|x| keeps rows from bleeding into each other).
            # Built on gpsimd (otherwise idle).  Only 16 rows: half chunks
            # scan both c-halves at once (16 segments), full chunks scan one
            # c-half per op (16 segments each) -- so every scan is a uniform
            # [P, 2048] op over the same shift tile, and the first scan only
            # waits for 16 memsets (~3 us) instead of 32.
            n_rows = 16
            shift = consts.tile([P, n_rows, W_DIM], FP16)
            for r in range(n_rows):
                nc.gpsimd.memset(shift[:, r, :], r * 16.0)
            shift_f = shift.rearrange("p r w -> p (r w)")

            y_prev = None
            hc_prev = 0
            row0 = 0
            for j, hc_j in enumerate(chunks):
                cpix = nb * hc_j * W_DIM
                if j == 0:
                    g = g0
                else:
                    g = gin.tile([P, NCH, nb, hc_j, W_DIM], FP16)
                    for ch in range(NCH):
                        nc.sync.dma_start(
                            out=g[:, ch],
                            in_=grid_t[ch][:, :, row0 : row0 + hc_j, :],
                        )
                g_f = g.rearrange("p c b h w -> p c (b h w)")

                # --- W-cummax: segmented scan along the flat (b, h, w) dim.
                x = xcp.tile([P, NCH, cpix], FP16)
                # The scan lowers to S2S2D2_STT, which has very few
                # sync-wait slots; absorb the cross-engine waits (g DMA,
                # x slot release) into a cheap DVE copy first.
                nc.vector.tensor_copy(x[:, :, 0:1], g_f[:, :, 0:1])
                if NCH * cpix == n_rows * W_DIM:
                    # half chunk: both c-halves in one 16-segment scan
                    _emit_segmax_2x(
                        nc,
                        out=x.rearrange("p c x -> p (c x)"),
                        in0=g_f.rearrange("p c x -> p (c x)"),
                        in1=shift_f,
                    )
                else:
                    # full chunk: one 16-segment scan per c-half
                    for ch in range(NCH):
                        _emit_segmax_2x(
                            nc, out=x[:, ch, :], in0=g_f[:, ch, :], in1=shift_f
                        )

                # --- H-cummax: row recurrence (c-halves x batches per op).
                y = ycp.tile([P, NCH, nb, hc_j, W_DIM], FP16)
                for hh in range(hc_j):
                    if j == 0 and hh == 0:
                        nc.vector.tensor_copy(y[:, :, :, 0, :], g[:, :, :, 0, :])
                    else:
                        prev = (
                            y[:, :, :, hh - 1, :]
                            if hh > 0
                            else y_prev[:, :, :, hc_prev - 1, :]
                        )
                        nc.vector.tensor_tensor(
                            y[:, :, :, hh, :], g[:, :, :, hh, :], prev, Alu.max
                        )
                y_prev = y
                hc_prev = hc_j
                y_f = y.rearrange("p c b h w -> p c (b h w)")

                # --- Matmul + bias. K order: xc0, xc1, yc0, yc1.
                # o has a 1-element pad: an ACT "touch" writes it so the
                # slot-release hazard (out-DMA) lands on the ACT clock
                # without overlapping the drains' writes (overlapping
                # same-engine WAW costs an extra sync-wait slot).
                o_raw = outp.tile([P, NCH * cpix + 1], FP16)
                nc.scalar.copy(
                    out=o_raw[:, NCH * cpix : NCH * cpix + 1],
                    in_=w_sb[:, 0, 0:1],
                )
                o = o_raw[:, 0 : NCH * cpix].rearrange("p (c x) -> p c x", c=NCH)
                for s in range(cpix // 1024):
                    for ch_out in range(NCH):
                        pt = psump.tile([P, 1024], FP32, tag="pt")
                        # N=1 dummy matmul: re-points the PSUM slot-release
                        # hazard (ACT) onto the PE clock, so real matmuls
                        # and the drain each carry a single sync wait (the
                        # LDW/AC structs allow only one).  The first one
                        # also absorbs the w_sb DMA wait.
                        nc.tensor.matmul(
                            pt[:, 0:1],
                            w_sb[:, 0, :],
                            w_sb[:, 0, 0:1],
                            start=True,
                            stop=True,
                        )
                        for pg in range(2):
                            lo = s * 1024 + pg * 512
                            for k in range(KT):
                                src = x if k < NCH else y_f
                                rhs = src[:, k % NCH, lo : lo + 512]
                                nc.tensor.matmul(
                                    pt[:, pg * 512 : (pg + 1) * 512],
                                    w_sb[:, ch_out * KT + k, :],
                                    rhs,
                                    start=(k == 0),
                                    stop=(k == KT - 1),
                                )
                        nc.scalar.activation(
                            out=o[:, ch_out, s * 1024 : (s + 1) * 1024],
                            in_=pt,
                            func=mybir.ActivationFunctionType.Identity,
                            bias=b_sb[:, ch_out : ch_out + 1],
                            scale=1.0,
                        )

                o_4d = o.rearrange("p c (b x) -> p c b x", b=nb)
                for ch in range(NCH):
                    nc.sync.dma_start(
                        out=out_t[ch][
                            :, :, row0 * W_DIM : (row0 + hc_j) * W_DIM
                        ],
                        in_=o_4d[:, ch],
                    )
                row0 += hc_j
    nc.compile()
    return nc


_built = {}


def _get_nc():
    if "nc" not in _built:
        _built["nc"] = build()
    return _built["nc"]


def make_in_maps(grid, Wm, bv):
    """Host-side shard + layout transform. Returns per-core input maps."""
    grid = np.asarray(grid, dtype=np.float32).astype(np.float16)
    Wm = np.asarray(Wm, dtype=np.float32).astype(np.float16)
    bv = np.asarray(bv, dtype=np.float32)
    # [f=P, ch*KT, c]: w_t[f, ch*KT+k, c] = W[k*P+f, ch*P+c] -- one
    # contiguous 2 KiB run per partition for the DMA.
    w_t = np.ascontiguousarray(
        Wm.reshape(KT, P, NCH, P).transpose(1, 2, 0, 3).reshape(P, NCH * KT, P)
    )
    b_t = np.ascontiguousarray(bv.reshape(NCH, P).T)
    in_maps = []
    for i in range(N_CORES):
        gc = grid[i * NB : (i + 1) * NB]  # [NB, H, W, C]
        # -> [ch, c, b, h, w]
        gt = np.ascontiguousarray(gc.transpose(3, 0, 1, 2)).reshape(
            NCH, P, NB, H, W_DIM
        )
        in_maps.append({"grid_t": gt, "w_t": w_t, "b_t": b_t})
    return in_maps


def assemble_output(results):
    """Per-core [NCH, P, NB, H*W] fp16 -> full [B, H, W, C] fp32."""
    outs = []
    for i in range(N_CORES):
        ot = results[i]["out_t"]
        oc = (
            ot.reshape(NCH, P, NB, H, W_DIM)
            .transpose(2, 3, 4, 0, 1)
            .reshape(NB, H, W_DIM, C)
            .astype(np.float32)
        )
        outs.append(oc)
    return np.ascontiguousarray(np.concatenate(outs, axis=0))


def run(inputs, **kwargs):
    """Run on hardware; returns (output, BassKernelResults)."""
    nc = _get_nc()
    in_maps = make_in_maps(inputs["grid"], inputs["W"], inputs["b"])
    res = run_bass_kernel_spmd(nc, in_maps, core_ids=list(range(N_CORES)), **kwargs)
    return assemble_output(res.results), res


def kernel(**inputs) -> np.ndarray:
    out, _ = run(inputs)
    return out



# revision 34
# speedup vs baseline: 1.1080x; 1.0153x over previous
"""Trainium2 Bass kernel for: cummax(W) ++ cummax(H) -> Linear(2C, C).

Reference semantics (shapes hardcoded):
    grid [16, 128, 128, 256] f32
    xc = cummax(grid, axis=2)   # along W
    yc = cummax(grid, axis=1)   # along H
    out = concat([xc, yc], -1) @ W[512, 256] + b[256]    # [16, 128, 128, 256]

Strategy: data-parallel over batch (2 batches / core on 8 cores).
Host pre-transposes grid to channels-first fp16 [c_half, c, b, h, w] so
on-chip tiles are [c(128 partitions), (c_half, b, h, w) free].  Both
per-core batches are processed together in h-row chunks (8 rows, with
half-size chunks at the ends to shrink the pipeline ramp and tail):
  - W-cummax: segmented max-scan (custom DVE op SEGMAX_ANT, row-shift
    trick) over the flat (b, h, w) dim -- 16 segments per op, with a
    hand-written 2X_1PORT uop program that runs 2 fp16 elems/cycle
    (2x the autogenerated 1x scan; see _segmax_uops_2x).
  - H-cummax: row-recurrence y[h] = max(y[h-1], g[h]) via fp16 DVE
    tensor_tensor at FD=512 (both c-halves x both batches per op).
  - Matmul: out[c_half, pix] += W_k[feat, c].T @ X_k[feat, pix], fp16
    operands, fp32 PSUM, 4 K-chunks (xc0, xc1, yc0, yc1), N=512 per MM.
    ~20 junk matmuls at kernel start warm the PE's HAM clock gate.
  - Bias is added during the ScalarE PSUM->SBUF copy; output is stored
    fp16 (host upcasts) to halve output DMA.
Everything fp16 on chip: monotone rounding commutes with cummax, and
the matmul accumulates in fp32, so rel err ~4e-4 vs the fp32 reference.
"""

import numpy as np

import concourse.tile as tile
from concourse import bacc, bass_isa, dve_ops, mybir
from concourse.bass_utils import run_bass_kernel_spmd
from concourse.dve_ops import DveOp
from concourse.dve_spec import AluOp, Spec, Src0, Src1, lower, scan
from concourse.dve_table_gen import dve_ver_for
from concourse.dve_uop import (
    AluInp,
    DelayInp,
    DveOpSpec,
    InpSel,
    OutPath,
    OutSel,
    Trigger,
    UopConfig,
    UopDpConfig,
)


def _segmax_uops_2x():
    """Hand-written 2X_1PORT uop program for the segmented max-scan.

    In 2X_1PORT mode the engine reads two consecutive fp16 elements per
    cycle per port (SRC_0/SRC_0_HI = data pair, SRC_1/SRC_1_HI = shift
    pair) and writes a packed pair (WR0_LO/WR0_HI).  Datapath per pair
    (xe, xo) with shifts (she, sho):
        A = xe + she; B = xo + sho          (blk0, blk1)
        C = max(A, B)                       (blk2, pair max)
        S = max(S, C)                       (blk3, running max, self-loop;
                                             its pre-update value S_prev is
                                             captured into a delay lane via
                                             DelayInp.CURR_ALU_OUT = "this
                                             block's flop, previous cycle")
        xc_e = max(S_prev, A)               (blk4)
        xc_o = max(xc_e, B)                 (blk5)
        OUT_e = xc_e - she                  (blk6 -> delay lane 0)
        OUT_o = xc_o - sho                  (blk7 -> ALU out)
    Write paths mirror the stock tensor_mask 2x program (slot 105):
    WR0_LO <- DELAY_0 (even), WR0_HI <- ALU_OUT (odd).
    """
    A, D, Op = AluInp, DelayInp, AluOp
    bs = [UopDpConfig() for _ in range(8)]
    # chains: c0=xe, c1=she, c2=xo, c3=sho, c4=MAX_NEG (seed const)
    bs[0].enable_alu(Op.ADD, A.PREV_DELAY_0, A.PREV_DELAY_1)
    bs[0].pass_through_delay(1, 2, 3, 4)
    bs[1].enable_alu(Op.ADD, A.PREV_DELAY_2, A.PREV_DELAY_3)
    bs[1].enable_delay_from_src(D.PREV_ALU_OUT, 0)  # c0 <- A
    bs[1].pass_through_delay(1, 3, 4)
    bs[2].enable_alu(Op.MAX, A.PREV_ALU_OUT, A.PREV_DELAY_0)  # C = max(B, A)
    bs[2].enable_delay_from_src(D.PREV_ALU_OUT, 2)  # c2 <- B
    bs[2].pass_through_delay(0, 1, 3, 4)
    bs[3].enable_alu(Op.MAX, A.CURR_ALU_OUT, A.PREV_ALU_OUT)  # S = max(S, C)
    bs[3].enable_delay_from_src(D.CURR_ALU_OUT, 4)  # c4 <- S_prev
    bs[3].pass_through_delay(0, 1, 2, 3)
    bs[4].enable_alu(Op.MAX, A.PREV_DELAY_4, A.PREV_DELAY_0)  # xc_e
    bs[4].pass_through_delay(1, 2, 3)
    bs[5].enable_alu(Op.MAX, A.PREV_ALU_OUT, A.PREV_DELAY_2)  # xc_o
    bs[5].enable_delay_from_src(D.PREV_ALU_OUT, 0)  # c0 <- xc_e
    bs[5].pass_through_delay(1, 3)
    bs[6].enable_alu(Op.SUBTRACT, A.PREV_DELAY_0, A.PREV_DELAY_1)  # OUT_e
    bs[6].enable_delay_from_src(D.PREV_ALU_OUT, 2)  # c2 <- xc_o
    bs[6].pass_through_delay(3)
    bs[7].enable_alu(Op.SUBTRACT, A.PREV_DELAY_2, A.PREV_DELAY_3)  # OUT_o
    bs[7].enable_delay_from_src(D.PREV_ALU_OUT, 0)  # c0 <- OUT_e

    inp = [
        InpSel.ZERO,
        InpSel.SRC_0,
        InpSel.SRC_1,
        InpSel.SRC_0_HI,
        InpSel.SRC_1_HI,
        InpSel.MAX_NEG,
        InpSel.ZERO,
        InpSel.ZERO,
    ]
    inp_en = [0, 1, 1, 1, 1, 1, 0, 0]
    steady = UopConfig(
        inp=inp,
        inp_enable=inp_en,
        out={
            OutPath.WR0_LO: OutSel.DELAY_0,
            OutPath.WR0_HI: OutSel.ALU_OUT,
            OutPath.WR1_LO: OutSel.ALU_OUT,
            OutPath.WR1_HI: OutSel.ALU_OUT,
        },
        out_enable={
            OutPath.WR0_LO: 1,
            OutPath.WR0_HI: 1,
            OutPath.WR1_LO: 0,
            OutPath.WR1_HI: 0,
        },
        require_inp0=1,
        require_inp1=1,
        trigger=(Trigger.SRC_TENSOR_DONE, Trigger.NONE, Trigger.NONE),
        next_uop=(0, 0, 0),
        repeat_count=0,
        datapath_config=bs,
    )
    # Seed: one cycle, no src consumed; MAX_NEG rides chain 4 to blk3 whose
    # BYPASS initialises the running-max flop (mirrors the 1x seed).
    sd = [UopDpConfig() for _ in range(8)]
    for k in range(3):
        sd[k].pass_through_delay(4)
    sd[3].enable_alu(Op.BYPASS, A.PREV_DELAY_4, A.PREV_DELAY_4)
    seed = UopConfig(
        inp=inp,
        inp_enable=inp_en,
        out={o: OutSel.ALU_OUT for o in OutPath},
        out_enable={o: 0 for o in OutPath},
        require_inp0=0,
        require_inp1=0,
        trigger=(Trigger.COUNT, Trigger.NONE, Trigger.NONE),
        next_uop=(1, 0, 0),
        repeat_count=1,
        datapath_config=sd,
    )
    return [seed, steady]


def _register_segmax():
    """Custom DVE op: segmented max-scan via the row-shift trick.

    body = scan(max, x + shift) - shift, where shift[row r] = r*BIG.  With
    BIG > 2*max|x| the running max can never cross a row boundary (later
    rows' shifted values strictly dominate), so one continuous scan yields
    an independent cummax per row.  The hand-written 2X_1PORT variant
    (see _segmax_uops_2x) processes 2 fp16 elems/cycle vs 1 for the
    autogenerated 1x program; the engine falls back to 1x whenever the
    runtime mem-pattern check fails, so 2x is purely a fast path.
    """
    name = "SEGMAX_ANT"
    if name in dve_ops._SUB_OPCODE_FOR_NAME:
        return next(o for o in dve_ops.OPS if o.name == name)
    ver = dve_ver_for("TRN2")
    row = dve_ops._CUSTOM_DVE_ROW_BASE + len(dve_ops.OPS)
    spec = Spec(
        body=scan(AluOp.MAX, Src0 + Src1) - Src1,
        reference=lambda in0, in1, s0, s1, imm2: (
            np.maximum.accumulate(
                (in0.astype(np.float32) + in1).reshape(in0.shape[0], -1), axis=1
            ).reshape(in0.shape)
            - in1
        ),
    )
    tmp = DveOpSpec(
        name=name,
        opcode=row,
        uops=lower(spec, ver=ver),
        uops_2x=_segmax_uops_2x(),
        rd1_en=True,
        perf_max=1,
    )
    tmp.validate(ver)
    op = DveOp(name, spec, subdim=False, uops_sha={ver: tmp.sha(ver)})
    # Seed the compile cache so DveOp.compile() returns the spec WITH the
    # 2x variant (it would otherwise re-lower the 1x-only program and fail
    # the sha pin).
    dve_ops._COMPILE_CACHE[(name, ver)] = tmp
    dve_ops.OPS.append(op)
    dve_ops.CUSTOM_DVE_SPECS[name] = spec
    dve_ops._SUB_OPCODE_FOR_NAME[name] = row
    return op


SEGMAX = _register_segmax()


def _emit_segmax_2x(nc, out, in0, in1):
    """nc.vector._custom_dve(SEGMAX, ...) with perf_max=1 in byte-36 so the
    engine may select the 2X_1PORT uop program (it re-checks dtype/stride/
    alignment at runtime and silently falls back to 1x)."""
    v = nc.vector
    if SEGMAX.name not in v.bass.m.ant_custom_dve_ops:
        v.bass.m.ant_custom_dve_ops = sorted(
            {*v.bass.m.ant_custom_dve_ops, SEGMAX.name}
        )
    shape = bass_isa.CustomDveShape.TTSS  # 2D in1 -> 1D free pattern
    isa_opcode = v.bass.isa.Opcode[
        f"NEURON_ISA_TPB_OPCODE_CUSTOM_DVE_ANT_{shape.slot()}"
    ].value
    zero = mybir.ImmediateValue(dtype=mybir.dt.float32, value=0.0)
    ins = [
        v.lower_ap(in0, for_isa=True, opt=True),
        v.lower_ap(in1, for_isa=True, opt=True),
        zero,
        zero,
    ]
    outs = [v.lower_ap(out, for_isa=True, opt=True)]
    return v.add_instruction(
        bass_isa.InstCustomDveAnt(
            name=v.bass.get_next_instruction_name(),
            op_name=SEGMAX.name,
            rd1_en=True,
            subdim=0,
            imm2=0.0,
            shape=shape,
            row=dve_ops.get_dve_sub_opcode(SEGMAX.name),
            isa_opcode=isa_opcode,
            ins=ins,
            outs=outs,
            perf_max=1,
        )
    )

B, H, W_DIM, C = 16, 128, 128, 256
N_CORES = 8
NB = B // N_CORES          # batches per core
P = 128                    # partitions
NCH = C // P               # channel halves (2)
KT = (2 * C) // P          # K chunks for the matmul (4)
HC = 8                     # h rows per streamed chunk (both batches)
NEG = -1.0e38

FP32 = mybir.dt.float32
FP16 = mybir.dt.float16


def build(nb=NB, h=H, hc=HC):
    """Build the per-core Bass program (same program on all cores)."""
    Alu = mybir.AluOpType
    # Chunk row counts: small chunks at both ends shrink the pipeline ramp
    # (first scan starts sooner) and tail (last chunk's matmul+drain+store
    # chain is shorter).  2-row chunks use a single 512-wide PSUM group.
    if h > 4 * hc:
        half = hc // 2
        q = hc // 4
        chunks = (
            [q, q, half] + [hc] * ((h - 2 * hc) // hc) + [half, q, q]
        )
    else:
        chunks = [hc] * (h // hc)
    assert sum(chunks) == h
    hw = h * W_DIM

    nc = bacc.Bacc("TRN2", target_bir_lowering=False, debug=False)
    grid_t = nc.dram_tensor(
        "grid_t", [NCH, P, nb, h, W_DIM], FP16, kind="ExternalInput"
    ).ap()
    # Weights pre-arranged on host as [f, ch*KT, c] so the DMA reads one
    # contiguous 2 KiB run per partition (256 B runs pay a 2x DMA penalty).
    w_t = nc.dram_tensor("w_t", [P, NCH * KT, P], FP16, kind="ExternalInput").ap()
    b_t = nc.dram_tensor("b_t", [P, NCH], FP32, kind="ExternalInput").ap()
    out_t = nc.dram_tensor(
        "out_t", [NCH, P, nb, hw], FP16, kind="ExternalOutput"
    ).ap()

    with tile.TileContext(nc) as tc:
        with (
            tc.tile_pool(name="consts", bufs=1) as consts,
            tc.tile_pool(name="gin", bufs=4) as gin,
            tc.tile_pool(name="xcp", bufs=3) as xcp,
            tc.tile_pool(name="ycp", bufs=3) as ycp,
            tc.tile_pool(name="outp", bufs=3) as outp,
            tc.tile_pool(name="psum", bufs=4, space="PSUM") as psump,
        ):
            # Chunk-0 grid DMA issued FIRST so the first scan's data is in
            # flight before the consts DMAs queue behind it.
            g0 = gin.tile([P, NCH, nb, chunks[0], W_DIM], FP16)
            for ch in range(NCH):
                nc.sync.dma_start(
                    out=g0[:, ch], in_=grid_t[ch][:, :, 0 : chunks[0], :]
                )
            # PE warm-up: the HAM clock gate holds the PE at ~half rate until
            # ~4 us of sustained activity.  Junk matmuls on a zeroed tile
            # warm it during the initial DMA wait so the real matmul stream
            # starts at full rate.
            warm = consts.tile([P, 512], FP16)
            nc.gpsimd.memset(warm, 0.0)
            wpt = psump.tile([P, 1024], FP32, tag="pt")
            for _ in range(20):
                nc.tensor.matmul(
                    wpt[:, 0:512], warm[:, 0:128], warm, start=True, stop=True
                )
            # Weights as 8 stationary [feat, c] tiles, indexed ch_out*KT + k.
            w_sb = consts.tile([P, NCH * KT, P], FP16)
            nc.sync.dma_start(out=w_sb, in_=w_t)
            b_sb = consts.tile([P, NCH], FP32)
            nc.sync.dma_start(out=b_sb, in_=b_t)
            # ACT touches b_sb once so later drains never carry the DMA wait
            # (the activation struct has a single sync-wait slot).
            b_scratch = consts.tile([P, NCH], FP32)
            nc.scalar.copy(out=b_scratch, in_=b_sb)
            # Row-shift tile for the custom segmented max-scan: row r gets
            # r*BIG (BIG > 2*max|x| keeps rows from bleeding into each other).
            # Built on gpsimd (otherwise idle).  Only 16 rows: half chunks
            # scan both c-halves at once (16 segments), full chunks scan one
            # c-half per op (16 segments each) -- so every scan is a uniform
            # [P, 2048] op over the same shift tile, and the first scan only
            # waits for 16 memsets (~3 us) instead of 32.
            n_rows = 16
            shift = consts.tile([P, n_rows, W_DIM], FP16)
            for r in range(n_rows):
                nc.gpsimd.memset(shift[:, r, :], r * 16.0)
            shift_f = shift.rearrange("p r w -> p (r w)")

            y_prev = None
            hc_prev = 0
            row0 = 0
            for j, hc_j in enumerate(chunks):
                cpix = nb * hc_j * W_DIM
                if j == 0:
                    g = g0
                else:
                    g = gin.tile([P, NCH, nb, hc_j, W_DIM], FP16)
                    for ch in range(NCH):
                        nc.sync.dma_start(
                            out=g[:, ch],
                            in_=grid_t[ch][:, :, row0 : row0 + hc_j, :],
                        )
                g_f = g.rearrange("p c b h w -> p c (b h w)")

                # --- W-cummax: segmented scan along the flat (b, h, w) dim.
                x = xcp.tile([P, NCH, cpix], FP16)
                # The scan lowers to S2S2D2_STT, which has very few
                # sync-wait slots; absorb the cross-engine waits (g DMA,
                # x slot release) into a cheap DVE copy first.
                nc.vector.tensor_copy(x[:, :, 0:1], g_f[:, :, 0:1])
                if NCH * cpix <= n_rows * W_DIM:
                    # small chunk: both c-halves in one <=16-segment scan
                    _emit_segmax_2x(
                        nc,
                        out=x.rearrange("p c x -> p (c x)"),
                        in0=g_f.rearrange("p c x -> p (c x)"),
                        in1=shift_f[:, 0 : NCH * cpix],
                    )
                else:
                    # full chunk: one 16-segment scan per c-half
                    for ch in range(NCH):
                        _emit_segmax_2x(
                            nc,
                            out=x[:, ch, :],
                            in0=g_f[:, ch, :],
                            in1=shift_f[:, 0:cpix],
                        )

                # --- H-cummax: row recurrence (c-halves x batches per op).
                y = ycp.tile([P, NCH, nb, hc_j, W_DIM], FP16)
                for hh in range(hc_j):
                    if j == 0 and hh == 0:
                        nc.vector.tensor_copy(y[:, :, :, 0, :], g[:, :, :, 0, :])
                    else:
                        prev = (
                            y[:, :, :, hh - 1, :]
                            if hh > 0
                            else y_prev[:, :, :, hc_prev - 1, :]
                        )
                        nc.vector.tensor_tensor(
                            y[:, :, :, hh, :], g[:, :, :, hh, :], prev, Alu.max
                        )
                y_prev = y
                hc_prev = hc_j
                y_f = y.rearrange("p c b h w -> p c (b h w)")

                # --- Matmul + bias. K order: xc0, xc1, yc0, yc1.
                # o has a 1-element pad: an ACT "touch" writes it so the
                # slot-release hazard (out-DMA) lands on the ACT clock
                # without overlapping the drains' writes (overlapping
                # same-engine WAW costs an extra sync-wait slot).
                o_raw = outp.tile([P, NCH * cpix + 1], FP16)
                nc.scalar.copy(
                    out=o_raw[:, NCH * cpix : NCH * cpix + 1],
                    in_=w_sb[:, 0, 0:1],
                )
                o = o_raw[:, 0 : NCH * cpix].rearrange("p (c x) -> p c x", c=NCH)
                npg = min(2, cpix // 512)  # 512-wide groups per PSUM tile
                gsz = npg * 512
                for s in range(cpix // gsz):
                    for ch_out in range(NCH):
                        pt = psump.tile([P, 1024], FP32, tag="pt")
                        # N=1 dummy matmul: re-points the PSUM slot-release
                        # hazard (ACT) onto the PE clock, so real matmuls
                        # and the drain each carry a single sync wait (the
                        # LDW/AC structs allow only one).  The first one
                        # also absorbs the w_sb DMA wait.
                        nc.tensor.matmul(
                            pt[:, 0:1],
                            w_sb[:, 0, :],
                            w_sb[:, 0, 0:1],
                            start=True,
                            stop=True,
                        )
                        for pg in range(npg):
                            lo = s * gsz + pg * 512
                            for k in range(KT):
                                src = x if k < NCH else y_f
                                rhs = src[:, k % NCH, lo : lo + 512]
                                nc.tensor.matmul(
                                    pt[:, pg * 512 : (pg + 1) * 512],
                                    w_sb[:, ch_out * KT + k, :],
                                    rhs,
                                    start=(k == 0),
                                    stop=(k == KT - 1),
                                )
                        nc.scalar.activation(
                            out=o[:, ch_out, s * gsz : (s + 1) * gsz],
                            in_=pt[:, 0:gsz],
                            func=mybir.ActivationFunctionType.Identity,
                            bias=b_sb[:, ch_out : ch_out + 1],
                            scale=1.0,
                        )

                o_4d = o.rearrange("p c (b x) -> p c b x", b=nb)
                for ch in range(NCH):
                    nc.sync.dma_start(
                        out=out_t[ch][
                            :, :, row0 * W_DIM : (row0 + hc_j) * W_DIM
                        ],
                        in_=o_4d[:, ch],
                    )
                row0 += hc_j
    nc.compile()
    return nc


_built = {}


def _get_nc():
    if "nc" not in _built:
        _built["nc"] = build()
    return _built["nc"]


def make_in_maps(grid, Wm, bv):
    """Host-side shard + layout transform. Returns per-core input maps."""
    grid = np.asarray(grid, dtype=np.float32).astype(np.float16)
    Wm = np.asarray(Wm, dtype=np.float32).astype(np.float16)
    bv = np.asarray(bv, dtype=np.float32)
    # [f=P, ch*KT, c]: w_t[f, ch*KT+k, c] = W[k*P+f, ch*P+c] -- one
    # contiguous 2 KiB run per partition for the DMA.
    w_t = np.ascontiguousarray(
        Wm.reshape(KT, P, NCH, P).transpose(1, 2, 0, 3).reshape(P, NCH * KT, P)
    )
    b_t = np.ascontiguousarray(bv.reshape(NCH, P).T)
    in_maps = []
    for i in range(N_CORES):
        gc = grid[i * NB : (i + 1) * NB]  # [NB, H, W, C]
        # -> [ch, c, b, h, w]
        gt = np.ascontiguousarray(gc.transpose(3, 0, 1, 2)).reshape(
            NCH, P, NB, H, W_DIM
        )
        in_maps.append({"grid_t": gt, "w_t": w_t, "b_t": b_t})
    return in_maps


def assemble_output(results):
    """Per-core [NCH, P, NB, H*W] fp16 -> full [B, H, W, C] fp32."""
    outs = []
    for i in range(N_CORES):
        ot = results[i]["out_t"]
        oc = (
            ot.reshape(NCH, P, NB, H, W_DIM)
            .transpose(2, 3, 4, 0, 1)
            .reshape(NB, H, W_DIM, C)
            .astype(np.float32)
        )
        outs.append(oc)
    return np.ascontiguousarray(np.concatenate(outs, axis=0))


def run(inputs, **kwargs):
    """Run on hardware; returns (output, BassKernelResults)."""
    nc = _get_nc()
    in_maps = make_in_maps(inputs["grid"], inputs["W"], inputs["b"])
    res = run_bass_kernel_spmd(nc, in_maps, core_ids=list(range(N_CORES)), **kwargs)
    return assemble_output(res.results), res


def kernel(**inputs) -> np.ndarray:
    out, _ = run(inputs)
    return out

